# revision 4
# baseline (speedup 1.0000x reference)
import sys

sys.path.insert(0, "/opt/trn_rl_repo")

import numpy as np

N = 100000
D = 32
E = 1600000
NCORES = 8
ROWS_PER_CORE = N // NCORES  # 12500
P = 128
WINDOWS = (ROWS_PER_CORE + P - 1) // P  # 98


def _host_pack(edge_row, edge_col, edge_val):
    """Per-core slot packing.

    Rows are count-sorted so each 128-row window holds rows with similar
    edge counts; window w then needs T_w = max slots in the window, with
    little padding. Slot g of row position p is one (col, val) pair; a
    bias pseudo-edge (col=N, val=1) is appended to every real row.

    Returns per-core (sorted_rows, slots list per window as (cols, vals)
    arrays of shape [P, T_w]) plus the global per-window slot counts.
    """
    edge_row = np.asarray(edge_row).astype(np.int64)
    edge_col = np.asarray(edge_col).astype(np.int64)
    edge_val = np.asarray(edge_val).astype(np.float32)

    core_of = edge_row // ROWS_PER_CORE
    per_core = []
    for c in range(NCORES):
        m = core_of == c
        r = edge_row[m] - c * ROWS_PER_CORE
        col = edge_col[m]
        val = edge_val[m]
        cnt = np.bincount(r, minlength=ROWS_PER_CORE)
        # sort rows by (cnt desc) for tight window packing
        order = np.argsort(-cnt, kind="stable")
        # position of each row in the sorted order
        pos_of_row = np.empty(ROWS_PER_CORE, dtype=np.int64)
        pos_of_row[order] = np.arange(ROWS_PER_CORE)
        # sort edges by their row's position so each row's edges are contiguous
        eorder = np.argsort(pos_of_row[r], kind="stable")
        r_s = pos_of_row[r[eorder]]
        col_s = col[eorder]
        val_s = val[eorder]
        # slot index within row: running counter, bias goes to slot cnt
        starts = np.zeros(ROWS_PER_CORE + 1, dtype=np.int64)
        starts[1:] = np.cumsum(cnt[order])
        slot_idx = np.arange(len(r_s)) - starts[r_s]
        per_core.append(
            dict(order=order, cnt_sorted=cnt[order], r_s=r_s, col_s=col_s,
                 val_s=val_s, slot_idx=slot_idx)
        )

    # global per-window T_w (same program across cores): max slots needed
    # (bias is added separately on-device, not as a pseudo-edge)
    T = np.zeros(WINDOWS, dtype=np.int64)
    for c in range(NCORES):
        cs = per_core[c]["cnt_sorted"]
        pad = np.zeros(WINDOWS * P, dtype=np.int64)
        pad[:ROWS_PER_CORE] = cs
        T = np.maximum(T, pad.reshape(WINDOWS, P).max(axis=1))
    T = np.maximum(T, 1)

    metas = []
    for c in range(NCORES):
        pc = per_core[c]
        tot = int(T.sum())
        cols = np.zeros((P, tot), dtype=np.int32)
        vals = np.zeros((P, tot), dtype=np.float32)
        offs = np.zeros(WINDOWS + 1, dtype=np.int64)
        offs[1:] = np.cumsum(T)
        # place real edges
        w_of = pc["r_s"] // P
        p_of = pc["r_s"] % P
        cidx = offs[w_of] + pc["slot_idx"]
        cols[p_of, cidx] = pc["col_s"].astype(np.int32)
        vals[p_of, cidx] = pc["val_s"]
        # pack cols+val-bits interleaved per window: [P, 2*T_w] blocks
        blocks = []
        for w in range(WINDOWS):
            a, b = offs[w], offs[w + 1]
            blocks.append(cols[:, a:b])
            blocks.append(vals[:, a:b].view(np.int32))
        metas.append(np.ascontiguousarray(np.concatenate(blocks, axis=1)))
    return per_core, metas, T


def _build_program(T):
    from concourse import bass, bacc, mybir
    import concourse.tile as tile

    nc = bacc.Bacc()
    wext = nc.declare_dram_parameter("wext", [N + 1, D], mybir.dt.float32,
                                     isOutput=False)
    tot2 = int(2 * T.sum())
    meta = nc.declare_dram_parameter("meta", [P, tot2], mybir.dt.int32,
                                     isOutput=False)
    biasrep = nc.declare_dram_parameter("biasrep", [P, D], mybir.dt.float32,
                                        isOutput=False)
    out = nc.declare_dram_parameter("out", [WINDOWS * P, D], mybir.dt.float32,
                                    isOutput=True)

    with tile.TileContext(nc) as tc:
        with tc.tile_pool(name="sbuf", bufs=3) as sbuf, \
             tc.tile_pool(name="msb", bufs=1) as msb:
            meta_sb = msb.tile([P, tot2], mybir.dt.int32)
            nc.sync.dma_start(out=meta_sb[:], in_=meta[:])
            bias_sb = msb.tile([P, D], mybir.dt.float32)
            nc.sync.dma_start(out=bias_sb[:], in_=biasrep[:])
            off = 0
            for w in range(WINDOWS):
                Tw = int(T[w])
                Tt = sbuf.tile([P, Tw * D], mybir.dt.float32, tag="T")
                for g in range(Tw):
                    nc.gpsimd.indirect_dma_start(
                        out=Tt[:, g * D:(g + 1) * D],
                        out_offset=None,
                        in_=wext[:],
                        in_offset=bass.IndirectOffsetOnAxis(
                            ap=meta_sb[:, off + g:off + g + 1], axis=0),
                    )
                valap = meta_sb[:, off + Tw:off + 2 * Tw].bitcast(
                    mybir.dt.float32)
                for g in range(Tw):
                    nc.vector.tensor_tensor(
                        out=Tt[:, g * D:(g + 1) * D],
                        in0=Tt[:, g * D:(g + 1) * D],
                        in1=valap[:, g:g + 1].to_broadcast([P, D]),
                        op=mybir.AluOpType.mult,
                    )
                # tree-reduce Tw chunks of D down to chunk 0
                n = Tw
                while n > 1:
                    lo = n // 2
                    hi = n - lo  # chunks [hi, n) add into [0, lo)
                    nc.vector.tensor_tensor(
                        out=Tt[:, 0:lo * D],
                        in0=Tt[:, 0:lo * D],
                        in1=Tt[:, hi * D:n * D],
                        op=mybir.AluOpType.add,
                    )
                    n = hi
                nc.vector.tensor_tensor(out=Tt[:, 0:D], in0=Tt[:, 0:D],
                                        in1=bias_sb[:],
                                        op=mybir.AluOpType.add)
                nc.sync.dma_start(out=out[w * P:(w + 1) * P, :],
                                  in_=Tt[:, 0:D])
                off += 2 * Tw
    nc.compile()
    global _LAST_NC
    _LAST_NC = nc
    return nc


_LAST_NC = None


def kernel(edge_row, edge_col, edge_val, weight, bias):
    from concourse.bass_utils import run_bass_kernel_spmd

    weight = np.asarray(weight).astype(np.float32)
    bias = np.asarray(bias).astype(np.float32)
    wext = np.concatenate([weight, bias[None, :]], axis=0)
    wext = np.ascontiguousarray(wext)

    per_core, metas, T = _host_pack(edge_row, edge_col, edge_val)
    nc = _build_program(T)

    biasrep = np.ascontiguousarray(np.tile(bias[None, :], (P, 1)))
    in_maps = [{"wext": wext, "meta": metas[c], "biasrep": biasrep}
               for c in range(NCORES)]
    res = run_bass_kernel_spmd(nc, in_maps, list(range(NCORES)))

    out_full = np.empty((N, D), dtype=np.float32)
    for c in range(NCORES):
        oc = res.results[c]["out"]  # [WINDOWS*P, D] in sorted-row order
        order = per_core[c]["order"]
        out_full[c * ROWS_PER_CORE + order, :] = oc[:ROWS_PER_CORE, :]
    return out_full



# revision 5
# speedup vs baseline: 1.0026x; 1.0026x over previous
import sys

sys.path.insert(0, "/opt/trn_rl_repo")

import numpy as np

N = 100000
D = 32
E = 1600000
NCORES = 8
ROWS_PER_CORE = N // NCORES  # 12500
P = 128
WINDOWS = (ROWS_PER_CORE + P - 1) // P  # 98


def _host_pack(edge_row, edge_col, edge_val):
    """Per-core slot packing.

    Rows are count-sorted so each 128-row window holds rows with similar
    edge counts; window w then needs T_w = max slots in the window, with
    little padding. Slot g of row position p is one (col, val) pair; a
    bias pseudo-edge (col=N, val=1) is appended to every real row.

    Returns per-core (sorted_rows, slots list per window as (cols, vals)
    arrays of shape [P, T_w]) plus the global per-window slot counts.
    """
    edge_row = np.asarray(edge_row).astype(np.int64)
    edge_col = np.asarray(edge_col).astype(np.int64)
    edge_val = np.asarray(edge_val).astype(np.float32)

    core_of = edge_row // ROWS_PER_CORE
    per_core = []
    for c in range(NCORES):
        m = core_of == c
        r = edge_row[m] - c * ROWS_PER_CORE
        col = edge_col[m]
        val = edge_val[m]
        cnt = np.bincount(r, minlength=ROWS_PER_CORE)
        # sort rows by (cnt desc) for tight window packing
        order = np.argsort(-cnt, kind="stable")
        # position of each row in the sorted order
        pos_of_row = np.empty(ROWS_PER_CORE, dtype=np.int64)
        pos_of_row[order] = np.arange(ROWS_PER_CORE)
        # sort edges by their row's position so each row's edges are contiguous
        eorder = np.argsort(pos_of_row[r], kind="stable")
        r_s = pos_of_row[r[eorder]]
        col_s = col[eorder]
        val_s = val[eorder]
        # slot index within row: running counter, bias goes to slot cnt
        starts = np.zeros(ROWS_PER_CORE + 1, dtype=np.int64)
        starts[1:] = np.cumsum(cnt[order])
        slot_idx = np.arange(len(r_s)) - starts[r_s]
        per_core.append(
            dict(order=order, cnt_sorted=cnt[order], r_s=r_s, col_s=col_s,
                 val_s=val_s, slot_idx=slot_idx)
        )

    # global per-window T_w (same program across cores): max slots needed
    # (bias is added separately on-device, not as a pseudo-edge)
    T = np.zeros(WINDOWS, dtype=np.int64)
    for c in range(NCORES):
        cs = per_core[c]["cnt_sorted"]
        pad = np.zeros(WINDOWS * P, dtype=np.int64)
        pad[:ROWS_PER_CORE] = cs
        T = np.maximum(T, pad.reshape(WINDOWS, P).max(axis=1))
    T = np.maximum(T, 1)

    metas = []
    for c in range(NCORES):
        pc = per_core[c]
        tot = int(T.sum())
        cols = np.zeros((P, tot), dtype=np.int32)
        vals = np.zeros((P, tot), dtype=np.float32)
        offs = np.zeros(WINDOWS + 1, dtype=np.int64)
        offs[1:] = np.cumsum(T)
        # place real edges
        w_of = pc["r_s"] // P
        p_of = pc["r_s"] % P
        cidx = offs[w_of] + pc["slot_idx"]
        cols[p_of, cidx] = pc["col_s"].astype(np.int32)
        vals[p_of, cidx] = pc["val_s"]
        # pack cols+val-bits interleaved per window: [P, 2*T_w] blocks
        blocks = []
        for w in range(WINDOWS):
            a, b = offs[w], offs[w + 1]
            blocks.append(cols[:, a:b])
            blocks.append(vals[:, a:b].view(np.int32))
        metas.append(np.ascontiguousarray(np.concatenate(blocks, axis=1)))
    return per_core, metas, T


def _build_program(T):
    from concourse import bass, bacc, mybir
    import concourse.tile as tile

    nc = bacc.Bacc()
    wext = nc.declare_dram_parameter("wext", [N + 1, D], mybir.dt.float32,
                                     isOutput=False)
    tot2 = int(2 * T.sum())
    meta = nc.declare_dram_parameter("meta", [P, tot2], mybir.dt.int32,
                                     isOutput=False)
    biasrep = nc.declare_dram_parameter("biasrep", [P, D], mybir.dt.float32,
                                        isOutput=False)
    out = nc.declare_dram_parameter("out", [WINDOWS * P, D], mybir.dt.float32,
                                    isOutput=True)

    with tile.TileContext(nc) as tc:
        with tc.tile_pool(name="sbuf", bufs=3) as sbuf, \
             tc.tile_pool(name="msb", bufs=1) as msb:
            meta_sb = msb.tile([P, tot2], mybir.dt.int32)
            # split the meta load so window 0's gathers only wait on its
            # own slice instead of the full 1.6MB transfer
            cut = int(2 * T[0])
            nc.sync.dma_start(out=meta_sb[:, 0:cut], in_=meta[:, 0:cut])
            nc.sync.dma_start(out=meta_sb[:, cut:], in_=meta[:, cut:])
            bias_sb = msb.tile([P, D], mybir.dt.float32)
            nc.sync.dma_start(out=bias_sb[:], in_=biasrep[:])
            off = 0
            for w in range(WINDOWS):
                Tw = int(T[w])
                Tt = sbuf.tile([P, Tw * D], mybir.dt.float32, tag="T")
                for g in range(Tw):
                    nc.gpsimd.indirect_dma_start(
                        out=Tt[:, g * D:(g + 1) * D],
                        out_offset=None,
                        in_=wext[:],
                        in_offset=bass.IndirectOffsetOnAxis(
                            ap=meta_sb[:, off + g:off + g + 1], axis=0),
                    )
                valap = meta_sb[:, off + Tw:off + 2 * Tw].bitcast(
                    mybir.dt.float32)
                for g in range(Tw):
                    nc.vector.tensor_tensor(
                        out=Tt[:, g * D:(g + 1) * D],
                        in0=Tt[:, g * D:(g + 1) * D],
                        in1=valap[:, g:g + 1].to_broadcast([P, D]),
                        op=mybir.AluOpType.mult,
                    )
                # tree-reduce Tw chunks of D down to chunk 0
                n = Tw
                while n > 1:
                    lo = n // 2
                    hi = n - lo  # chunks [hi, n) add into [0, lo)
                    nc.vector.tensor_tensor(
                        out=Tt[:, 0:lo * D],
                        in0=Tt[:, 0:lo * D],
                        in1=Tt[:, hi * D:n * D],
                        op=mybir.AluOpType.add,
                    )
                    n = hi
                nc.vector.tensor_tensor(out=Tt[:, 0:D], in0=Tt[:, 0:D],
                                        in1=bias_sb[:],
                                        op=mybir.AluOpType.add)
                nc.sync.dma_start(out=out[w * P:(w + 1) * P, :],
                                  in_=Tt[:, 0:D])
                off += 2 * Tw
    nc.compile()
    global _LAST_NC
    _LAST_NC = nc
    return nc


_LAST_NC = None


def kernel(edge_row, edge_col, edge_val, weight, bias):
    from concourse.bass_utils import run_bass_kernel_spmd

    weight = np.asarray(weight).astype(np.float32)
    bias = np.asarray(bias).astype(np.float32)
    wext = np.concatenate([weight, bias[None, :]], axis=0)
    wext = np.ascontiguousarray(wext)

    per_core, metas, T = _host_pack(edge_row, edge_col, edge_val)
    nc = _build_program(T)

    biasrep = np.ascontiguousarray(np.tile(bias[None, :], (P, 1)))
    in_maps = [{"wext": wext, "meta": metas[c], "biasrep": biasrep}
               for c in range(NCORES)]
    res = run_bass_kernel_spmd(nc, in_maps, list(range(NCORES)))

    out_full = np.empty((N, D), dtype=np.float32)
    for c in range(NCORES):
        oc = res.results[c]["out"]  # [WINDOWS*P, D] in sorted-row order
        order = per_core[c]["order"]
        out_full[c * ROWS_PER_CORE + order, :] = oc[:ROWS_PER_CORE, :]
    return out_full

